# revision 8
# baseline (speedup 1.0000x reference)
"""Trainium2 Bass kernel for nn_AttentionHead_Hybrid1 (quantum-inspired attention head).

Computation (per batch b of a [B=64, S=1024, D=64] input):
    n_i   = ||x_i||;  u_i = x_i / n_i
    W     = givens_orthogonal(phi)                (tiny, sequential -> host)
    A     = (n_i n_j) (u_i^T W^T u_j)^2           (S x S scores)
    V     = x Wv^T + bv
    out   = LayerNorm(softmax(A/sqrt(D)) V + x)

Kernel strategy (data-parallel over batch, 8 batches per NeuronCore):
  * Fold norms into the score matmul:  ut_i = x_i / sqrt(n_i),
    G^T = (ut W) ut^T, so A^T = G^T * G^T elementwise.
  * Scores computed directly in transposed [j, i] layout so the PV matmul
    needs no transpose of the softmax matrix.
  * Softmax denominator comes out of the PV matmul via an appended
    all-ones column of V (V2 = V'' * sqrt(n_j) restores the true V rows,
    making column 64 exactly 1).
  * No max-subtraction needed: max exponent of A/sqrt(D) is ~3.
  * The softmax division is eliminated entirely: LayerNorm is invariant to
    per-row scaling, so we feed it h' = (P V) + denom * x and scale the
    variance epsilon by denom^2.
  * Deferred-PV pipeline with a FINE-GRAINED interleaved PE stream:
    window k alternates score-chunk matmuls of batch k with PV matmuls of
    batch k-1 and ut-transposes of batch k+1, chunk by chunk.  The PE
    never sits in a long drain-wait (score chunks are V/S-drain paced) and
    the HAM clock gate stays at K=8/8 = 2.4 GHz.
  * Warm-up and tail transposes use REAL matmuls (transpose-mode does not
    register as PE-busy for the HAM clock gate).
  * Squares of G run from PSUM split across ScalarE (direct Square
    activation, doubling as the drain) and staged V/GpSimd multiplies;
    exp runs as one giant activation per batch.
  * All PSUM matmul outputs are padded to 2KB-bank-aligned chunk strides
    (a [*,NC,65] fp32 chunk layout silently corrupts chunk 7 on HW).
  * The activation-table map is restricted so there is exactly one ACT
    table load in the whole kernel.
"""

import math
import sys

import numpy as np

sys.path.insert(0, "/opt/trn_rl_repo")

import concourse.bass as bass
import concourse.bacc as bacc
import concourse.tile as tile
from concourse import mybir
from concourse.bass_utils import run_bass_kernel_spmd

try:
    import ml_dtypes
    BF16_NP = ml_dtypes.bfloat16
except ImportError:  # pragma: no cover
    BF16_NP = None

F32 = mybir.dt.float32
BF16 = mybir.dt.bfloat16
AX = mybir.AxisListType.X
OP = mybir.AluOpType
AF = mybir.ActivationFunctionType

B, S, D = 64, 1024, 64
NCORES = 8
NB = B // NCORES          # batches per core
NC = S // 128             # 128-row chunks per batch
DA = D + 1                # V augmented with ones column
INV_SQRT_D = 1.0 / math.sqrt(D)
LN_EPS = 1e-5

# engine for each score chunk's square: "se" squares straight out of PSUM
# (doubling as the drain); "dve"/"gps" need a V-engine staging copy first.
SQ_ENGINE = ["se", "dve", "gps", "se", "dve", "se", "gps", "dve"]
N_WARMUP_MM = 40          # real PE warm-up matmuls during initial DMA

_ACT_SET = "natural_log_exp_and_others"


def _patch_act_tables():
    """Make every activation resolve to one table set (it contains every
    function this kernel uses), so the compiled stream has exactly one
    ACT_TABLE_LOAD instead of ping-ponging between per-anchor sets."""
    from concourse import hw_specs

    if getattr(bacc, "_act_tables_patched", False):
        return
    orig = hw_specs.get_activation_tables

    def patched(arch):
        tabs = orig(arch)
        return {
            name: (funcs if name == _ACT_SET else set())
            for name, funcs in tabs.items()
        }

    bacc.get_activation_tables = patched
    bacc._act_tables_patched = True


def _givens_orthogonal(phi: np.ndarray, d: int) -> np.ndarray:
    pairs = [(i, i + 1) for i in range(d - 1)] + [(i, i + 1) for i in range(d - 3, -1, -1)]
    W = np.eye(d, dtype=np.float64)
    p = phi.astype(np.float64)
    for k, (i, j) in enumerate(pairs):
        c, s = np.cos(p[k]), np.sin(p[k])
        ri, rj = W[i].copy(), W[j].copy()
        W[i] = c * ri + s * rj
        W[j] = -s * ri + c * rj
    return W.astype(np.float32)


def _bcast_inner(ap, n):
    """[P, NC] -> [P, NC, n] with stride-0 inner dim."""
    return ap.unsqueeze(2).broadcast_to((ap.shape[0], ap.shape[1], n))


def _build_nc() -> bass.Bass:
    _patch_act_tables()
    nc = bacc.Bacc("TRN2", target_bir_lowering=False, debug=False, num_devices=NCORES)

    x_d = nc.dram_tensor("x", [NB, S, D], F32, kind="ExternalInput").ap()
    w_d = nc.dram_tensor("wg", [D, D], BF16, kind="ExternalInput").ap()
    wv_d = nc.dram_tensor("wv_aug", [DA, DA], BF16, kind="ExternalInput").ap()
    idb_d = nc.dram_tensor("ident_b", [128, 128], BF16, kind="ExternalInput").ap()
    idf_d = nc.dram_tensor("ident_f", [128, 128], F32, kind="ExternalInput").ap()
    out_d = nc.dram_tensor("out", [NB, S, D], F32, kind="ExternalOutput").ap()

    with tile.TileContext(nc) as tc:
        with (
            tc.tile_pool(name="const", bufs=1) as constp,
            tc.tile_pool(name="xin", bufs=4) as xin,
            tc.tile_pool(name="prep", bufs=2) as prep,
            tc.tile_pool(name="vsbp", bufs=3) as vsbp,
            tc.tile_pool(name="stats", bufs=2) as stats,
            tc.tile_pool(name="sqp", bufs=2) as sqp,
            tc.tile_pool(name="lnp", bufs=2) as lnp,
            tc.tile_pool(name="pg", bufs=2, space="PSUM") as pg,
            tc.tile_pool(name="pprep", bufs=1, space="PSUM") as pprep,
            tc.tile_pool(name="po", bufs=1, space="PSUM") as pop,
        ):
            w_sb = constp.tile([D, D], BF16)
            nc.sync.dma_start(w_sb, w_d)
            wv_sb = constp.tile([DA, DA], BF16)
            nc.sync.dma_start(wv_sb, wv_d)
            ident_b = constp.tile([128, 128], BF16)
            nc.sync.dma_start(ident_b, idb_d)
            ident_f = constp.tile([128, 128], F32)
            nc.sync.dma_start(ident_f, idf_d)

            # ---- PE warm-up: REAL matmuls so HAM ungates to 2.4 GHz -------
            warm = pg.tile([128, 1024], F32, tag="g")
            for k in range(N_WARMUP_MM):
                nc.tensor.matmul(warm[0:128, 0:128], ident_b, ident_b,
                                 start=True, stop=True)

            def emit_load(b):
                x_sb = xin.tile([128, NC, D], F32, tag="x")
                nc.sync.dma_start(x_sb, x_d[b].rearrange("(c p) d -> p c d", p=128))
                return x_sb

            def emit_stats(b, x_sb):
                """norms + ut rows for batch b (no PE)."""
                xsq = stats.tile([128, NC, D], F32, tag="xsq")
                nc.gpsimd.tensor_mul(xsq, x_sb, x_sb)
                nsq = stats.tile([128, NC], F32, tag="nsq")
                nc.vector.reduce_sum(nsq, xsq, axis=AX)
                lnn = stats.tile([128, NC], F32, tag="lnn")
                nc.scalar.activation(lnn, nsq, AF.Ln)
                s_t = stats.tile([128, NC], F32, tag="s")
                nc.scalar.activation(s_t, lnn, AF.Exp, scale=-0.25)
                rtn = stats.tile([128, NC], F32, tag="rtn")
                nc.scalar.activation(rtn, lnn, AF.Exp, scale=0.25)

                ut = prep.tile([128, NC, DA], BF16, tag="ut")
                nc.gpsimd.tensor_mul(ut[:, :, 0:D], x_sb, _bcast_inner(s_t, D))
                nc.gpsimd.tensor_copy(ut[:, :, D], s_t)
                return ut, rtn

            # ---------------- per-window emission helpers ------------------
            def window(k, state):
                """Window k: scores(k) x PV(k-1) x transposes(k+1) interleaved
                on the PE; then zT/V2 prep for k+1 and tail of k-1."""
                have_scores = k < NB
                have_prev = k >= 1
                have_next = k + 1 < NB

                if k + 2 < NB:
                    state["x"][k + 2] = emit_load(k + 2)
                st = emit_stats(k + 1, state["x"][k + 1]) if have_next else None
                if st is not None:
                    ut_n, rtn_n = st
                    ptall = pprep.tile([DA, S], BF16, tag="prep")

                if have_scores:
                    utT, zT, v_sb = state["prep"][k]
                    sq = sqp.tile([128, NC, S], BF16, tag="sq")
                if have_prev:
                    bp = k - 1
                    v_sb_p = state["prep"][bp][2]
                    p_t_p = state["pt"].pop(bp)
                    po = pop.tile([DA, S], F32, tag="po")

                # ---- interleaved PE stream over chunks -------------------
                for jc in range(NC):
                    if have_scores:
                        gp = pg.tile([128, S], F32, tag="g")
                        for h in range(2):
                            nc.tensor.matmul(
                                gp[:, h * 512:(h + 1) * 512],
                                zT[:, jc * 128:(jc + 1) * 128],
                                utT[0:D, h * 512:(h + 1) * 512],
                                start=True, stop=True,
                            )
                        eng = SQ_ENGINE[jc]
                        if eng == "se":
                            nc.scalar.activation(sq[:, jc, :], gp, AF.Square)
                        else:
                            gc = sqp.tile([128, S], BF16, tag="gc")
                            nc.vector.tensor_copy(gc, gp)
                            if eng == "gps":
                                nc.gpsimd.tensor_mul(sq[:, jc, :], gc, gc)
                            else:
                                nc.vector.tensor_mul(sq[:, jc, :], gc, gc)
                    if have_prev:
                        for h in range(2):
                            nc.tensor.matmul(
                                po[:, h * 512:(h + 1) * 512], v_sb_p[:, jc, :],
                                p_t_p[:, jc, h * 512:(h + 1) * 512],
                                start=(jc == 0), stop=(jc == NC - 1),
                            )
                    if st is not None:
                        nc.tensor.transpose(
                            ptall[:, jc * 128:(jc + 1) * 128], ut_n[:, jc, :],
                            ident_b,
                        )

                # ---- exp(k): one giant activation ------------------------
                if have_scores:
                    p_t = sqp.tile([128, NC, S], BF16, tag="p")
                    nc.scalar.activation(p_t, sq, AF.Exp, scale=INV_SQRT_D)
                    state["pt"][k] = p_t

                # ---- tail(k-1): drain po, transpose back, LayerNorm ------
                if have_prev:
                    o_sb = lnp.tile([DA, S], F32, tag="o")
                    nc.vector.tensor_copy(o_sb, po)

                # ---- prep matmuls for k+1 (zT, V2) -----------------------
                if st is not None:
                    utT_n = prep.tile([DA, S], BF16, tag="utT")
                    nc.vector.tensor_copy(utT_n, ptall)
                    zpall = pprep.tile([D, S], F32, tag="prep")
                    for h in range(2):
                        nc.tensor.matmul(
                            zpall[:, h * 512:(h + 1) * 512], w_sb,
                            utT_n[0:D, h * 512:(h + 1) * 512],
                            start=True, stop=True,
                        )
                    zT_n = prep.tile([D, S], BF16, tag="zT")
                    nc.vector.tensor_copy(zT_n, zpall)

                if have_prev:
                    # htall reuses the po psum slot; inner dim padded to 128
                    # (PSUM bank alignment).  REGULAR matmuls (HAM-visible).
                    htall = pop.tile([128, NC, 128], F32, tag="po")
                    for c in range(NC):
                        nc.tensor.matmul(
                            htall[:, c, 0:DA],
                            o_sb[:, c * 128:(c + 1) * 128],
                            ident_f[0:DA, 0:DA],
                            start=True, stop=True,
                        )

                if st is not None:
                    # V2 padded to 128 (PSUM bank alignment)
                    vpall = pg.tile([128, NC, 128], F32, tag="g")
                    for c in range(NC):
                        nc.tensor.matmul(
                            vpall[:, c, 0:DA],
                            utT_n[:, c * 128:(c + 1) * 128], wv_sb,
                            start=True, stop=True,
                        )
                    v_sb_n = vsbp.tile([128, NC, DA], BF16, tag="v")
                    nc.vector.tensor_mul(v_sb_n, vpall[:, :, 0:DA],
                                         _bcast_inner(rtn_n, DA))
                    state["prep"][k + 1] = (utT_n, zT_n, v_sb_n)

                if have_prev:
                    emit_ln_tail(bp, state["x"].pop(bp), htall)
                    del state["prep"][bp]

            def emit_ln_tail(b, x_sb, htall):
                dn = lnp.tile([128, NC], F32, tag="dn")
                nc.vector.tensor_copy(dn, htall[:, :, D])
                # h' = denom * x + attn_numer  (LayerNorm scale-invariance)
                xd = lnp.tile([128, NC, D], F32, tag="xd")
                nc.gpsimd.tensor_mul(xd, x_sb, _bcast_inner(dn, D))
                hp = lnp.tile([128, NC, D], F32, tag="hp")
                nc.vector.tensor_add(hp, xd, htall[:, :, 0:D])

                # ---- LayerNorm stats over D: sums + sum-of-squares --------
                m1 = lnp.tile([128, NC], F32, tag="m1")
                nc.vector.reduce_sum(m1, hp, axis=AX)
                hsq = lnp.tile([128, NC, D], F32, tag="hsq")
                nc.gpsimd.tensor_mul(hsq, hp, hp)
                m2 = lnp.tile([128, NC], F32, tag="m2")
                nc.vector.reduce_sum(m2, hsq, axis=AX)
                mu = lnp.tile([128, NC], F32, tag="mu")
                nc.vector.tensor_scalar_mul(mu, m1, 1.0 / D)
                t1 = lnp.tile([128, NC], F32, tag="t1")
                nc.gpsimd.tensor_mul(t1, mu, m1)
                vv = lnp.tile([128, NC], F32, tag="vv")
                nc.gpsimd.tensor_sub(vv, m2, t1)
                dn2 = lnp.tile([128, NC], F32, tag="dn2")
                nc.gpsimd.tensor_mul(dn2, dn, dn)
                # u = vv + (D*eps) * dn2 ; var + eps*dn^2 = u / D
                u_t = lnp.tile([128, NC], F32, tag="u")
                nc.vector.scalar_tensor_tensor(
                    out=u_t, in0=dn2, scalar=float(D) * LN_EPS, in1=vv,
                    op0=OP.mult, op1=OP.add,
                )
                lnv = stats.tile([128, NC], F32, tag="lnv")
                nc.scalar.activation(lnv, u_t, AF.Ln, scale=1.0 / D)
                rstd = stats.tile([128, NC], F32, tag="rstd")
                nc.scalar.activation(rstd, lnv, AF.Exp, scale=-0.5)

                hm = lnp.tile([128, NC, D], F32, tag="hm")
                nc.gpsimd.tensor_tensor(
                    out=hm, in0=hp, in1=_bcast_inner(mu, D), op=OP.subtract,
                )
                o_rows = lnp.tile([128, NC, D], F32, tag="orows")
                nc.gpsimd.tensor_mul(o_rows, hm, _bcast_inner(rstd, D))
                nc.sync.dma_start(
                    out_d[b].rearrange("(c p) d -> p c d", p=128), o_rows
                )

            # ---------------- run the pipeline -----------------------------
            state = {"x": {0: emit_load(0), 1: emit_load(1)},
                     "prep": {}, "pt": {}}
            # prologue prep for batch 0 (PE transposes not interleaved yet)
            ut0, rtn0 = emit_stats(0, state["x"][0])
            ptall0 = pprep.tile([DA, S], BF16, tag="prep")
            for c in range(NC):
                nc.tensor.transpose(ptall0[:, c * 128:(c + 1) * 128],
                                    ut0[:, c, :], ident_b)
            utT0 = prep.tile([DA, S], BF16, tag="utT")
            nc.vector.tensor_copy(utT0, ptall0)
            zpall0 = pprep.tile([D, S], F32, tag="prep")
            for h in range(2):
                nc.tensor.matmul(zpall0[:, h * 512:(h + 1) * 512], w_sb,
                                 utT0[0:D, h * 512:(h + 1) * 512],
                                 start=True, stop=True)
            zT0 = prep.tile([D, S], BF16, tag="zT")
            nc.vector.tensor_copy(zT0, zpall0)
            vpall0 = pg.tile([128, NC, 128], F32, tag="g")
            for c in range(NC):
                nc.tensor.matmul(vpall0[:, c, 0:DA],
                                 utT0[:, c * 128:(c + 1) * 128], wv_sb,
                                 start=True, stop=True)
            v_sb0 = vsbp.tile([128, NC, DA], BF16, tag="v")
            nc.vector.tensor_mul(v_sb0, vpall0[:, :, 0:DA],
                                 _bcast_inner(rtn0, DA))
            state["prep"][0] = (utT0, zT0, v_sb0)

            for k in range(NB + 1):
                window(k, state)
    nc.compile()
    return nc


_CACHED = None


def _get_nc():
    global _CACHED
    if _CACHED is None:
        _CACHED = _build_nc()
    return _CACHED


def _to_bf16(a: np.ndarray) -> np.ndarray:
    if BF16_NP is not None:
        return a.astype(BF16_NP)
    u = np.ascontiguousarray(a.astype(np.float32)).view(np.uint32)
    r = ((u >> 16) & 1).astype(np.uint32)
    return (((u + 0x7FFF + r) >> 16).astype(np.uint16)).view(np.uint16)


def kernel(x: np.ndarray, Wv: np.ndarray, bv: np.ndarray, phi: np.ndarray) -> np.ndarray:
    x = np.ascontiguousarray(np.asarray(x, np.float32))
    Wv = np.asarray(Wv, np.float32)
    bv = np.asarray(bv, np.float32)
    phi = np.asarray(phi, np.float32)
    assert x.shape == (B, S, D), x.shape

    wg = _givens_orthogonal(phi, D)
    wv_aug = np.zeros((DA, DA), np.float32)
    wv_aug[0:D, 0:D] = Wv.T
    wv_aug[D, 0:D] = bv
    wv_aug[D, D] = 1.0

    nc = _get_nc()
    in_maps = [
        {
            "x": np.ascontiguousarray(x[c * NB:(c + 1) * NB]),
            "wg": _to_bf16(wg),
            "wv_aug": _to_bf16(wv_aug),
            "ident_b": _to_bf16(np.eye(128, dtype=np.float32)),
            "ident_f": np.eye(128, dtype=np.float32),
        }
        for c in range(NCORES)
    ]
    res = run_bass_kernel_spmd(nc, in_maps, list(range(NCORES)))
    out = np.concatenate([res.results[c]["out"] for c in range(NCORES)], axis=0)
    return out.astype(np.float32)


if __name__ == "__main__":
    rng = np.random.default_rng(0)
    x = rng.standard_normal((B, S, D)).astype(np.float32)
    Wv = (rng.standard_normal((D, D)) / math.sqrt(D)).astype(np.float32)
    bv = (rng.standard_normal(D) * 0.01).astype(np.float32)
    phi = rng.uniform(0, 2 * math.pi, 2 * D - 3).astype(np.float32)
    y = kernel(x=x, Wv=Wv, bv=bv, phi=phi)
    print("out", y.shape, y.dtype, np.abs(y).mean())
